# revision 41
# baseline (speedup 1.0000x reference)
"""Single-head causal attention (B=4, T=2048, C=2048, H=128) on 8 TRN2 cores.

Partial-attention sharding, no inter-core communication: 2 cores per batch.
Core (2b + par) owns the 8 key tiles {128*(2m+par)} of batch b and computes
  - K^T, V^T for its 1024 own key positions only,
  - Q^T for ALL 2048 query positions of the batch,
  - partial attention:   ot = sum_{own k} exp(s) * v,  den = sum_{own k} exp(s)
over every query. The host adds the two cores' partials per batch and divides
(softmax sums commute; the max-shift is skipped since |s| < ~6).

Per-core x.T is column-permuted to [own tiles | sib tiles]; with that order
the kernel is SPMD-identical:
  - K/V project from cols [0, 1024), Q from all cols,
  - attention key tile j covers query cols [128j, 1024) of BOTH halves:
    own-half diag block gets the triangular mask, sib-half diag block a
    per-core scalar (par=0 -> 1.0, par=1 -> 0.0).

Schedule (from trace analysis; phase 1 used to be PE-bound and serial):
  phase 1a projects K/V/Q-OWN only (6 matmuls per c-tile) and needs just
  the own-half columns of x (4.2MB instead of 8.4MB), so it ends ~9us
  earlier; the Q-SIB projection (32 matmuls) is deferred into attention
  half0's ACT-bound gaps, with the sib-half of x streaming lazily in the
  background. Attention half1 then uses the freshly copied Q-sib.

  PSUM as four 2-bank [128,1024] fp32 tags A-D:
    1a: K->A, V->B, Qown->C.  Transposes on B/D.  Qsib accumulates in D
    during half0 iters 0-3 (copied out at iter 4).
    half0: O on B, S rotates [A, C], dens on A.
    half1: O on C, S rotates [D, A, B], dens on D, A.
  den = one ones-matmul per 512 cols over a bf16 A_sum accumulated on the
  DVE. S is emitted 2 key tiles ahead of O; masks sit ahead of the
  deferred A_sum adds on the DVE queue. exp is one wide ACT instruction
  per key tile. Outputs are bf16; O cols [0,512) ship at iter 4.
"""

import numpy as np
import ml_dtypes

B, T, C, H = 4, 2048, 2048, 128
P = 128                 # tile edge
NCT = C // P            # 16 contraction c-tiles
NKT = 8                 # own key tiles per core
NQ = 2048               # query cols per core (own 1024 | sib 1024)
N_CORES = 8
SCALE = float(H) ** -0.5
BF16 = ml_dtypes.bfloat16

WQUARTER = 4            # weight c-tiles per DMA chunk (4 chunks per tensor)
# own-half x chunks: (c-tile list, ring); weight quarter q ships just
# before the x chunk at index WQ_BEFORE[g] (quarter 0 before everything)
XSCHED = [
    ([0], "scalar"), ([1], "sync"), ([2, 3], "scalar"), ([4, 5], "sync"),
    ([6, 7], "scalar"), ([8, 9], "sync"), ([10, 11], "scalar"),
    ([12, 13], "sync"), ([14], "scalar"), ([15], "scalar"),
]
WQ_BEFORE = {3: (1, "sync"), 5: (2, "sync"), 7: (3, "sync")}
# sib-half x chunks (4 c-tiles each), needed only by the deferred Q-sib
# matmuls in attention half0
SIBSCHED = [("scalar", 0), ("sync", 1), ("scalar", 2), ("sync", 3)]

_cache = {}


def _build():
    import concourse.bass as bass
    import concourse.mybir as mybir
    import concourse.tile as tile
    from concourse import bacc
    from concourse.masks import make_identity, make_upper_triangular

    dt = mybir.dt
    nc = bacc.Bacc(
        "TRN2",
        target_bir_lowering=False,
        debug=False,
        enable_asserts=False,
        num_devices=N_CORES,
    )

    xkvT = nc.dram_tensor("xkvT", [C, T], dt.bfloat16, kind="ExternalInput").ap()
    w_d = {
        (nm, q): nc.dram_tensor(
            f"{nm}q{q}", [P, WQUARTER, H], dt.bfloat16, kind="ExternalInput"
        ).ap()
        for nm in ("wq", "wk", "wv")
        for q in range(NCT // WQUARTER)
    }
    # sib-half diag block allowed: 1.0 on par=0 cores, 0.0 on par=1 cores
    odd_d = nc.dram_tensor("odd", [P, 1], dt.float32, kind="ExternalInput").ap()
    ot_d = nc.dram_tensor("ot", [H, NQ], dt.bfloat16, kind="ExternalOutput").ap()
    den_d = nc.dram_tensor("den", [1, NQ], dt.float32, kind="ExternalOutput").ap()

    with tile.TileContext(nc) as tc:
        with (
            tc.tile_pool(name="persist", bufs=1) as persist,
            tc.tile_pool(name="ephem", bufs=8) as ephem,
            tc.tile_pool(name="outp", bufs=2) as outp,
            tc.tile_pool(name="psum", bufs=1, space="PSUM") as psum,
        ):
            # PSUM = four 2-bank [128,1024] fp32 tiles, tags A..D
            def pbank(tag, shape=(P, 1024), dtype=dt.float32, name=None):
                return psum.tile(
                    list(shape), dtype, tag=f"pp{tag}", name=name or f"pp{tag}"
                )

            wq_sb = persist.tile([P, NCT, H], dt.bfloat16)
            wk_sb = persist.tile([P, NCT, H], dt.bfloat16)
            wv_sb = persist.tile([P, NCT, H], dt.bfloat16)
            odd_sb = persist.tile([P, 1], dt.float32)
            xg_sb = [
                persist.tile([P, len(cts), 1024], dt.bfloat16, name=f"xg{g}")
                for g, (cts, _) in enumerate(XSCHED)
            ]
            xs_sb = [
                persist.tile([P, WQUARTER, 1024], dt.bfloat16, name=f"xs{g}")
                for g in range(len(SIBSCHED))
            ]
            k_sb = persist.tile([P, P * NKT], dt.bfloat16)   # K^T own [h, 1024]
            vt_sb = persist.tile([P, P * NKT], dt.bfloat16)  # V^T own [h, 1024]
            v_sb = persist.tile([P, NKT, H], dt.bfloat16)    # V tiles [k, h]
            q_sb = persist.tile([P, NQ], dt.bfloat16)        # Q^T [h, 2048]
            # A_sum in bf16: half the DVE add cost, den matmul reads it
            # directly (den rel err ~0.5%, well inside the 2e-2 gate)
            asum = [
                persist.tile([P, 1024], dt.bfloat16, name=f"asum{h}") for h in (0, 1)
            ]
            ident = persist.tile([P, P], dt.bfloat16)
            tri = persist.tile([P, P], dt.bfloat16)          # 1 where k <= q
            ones_sb = persist.tile([P, 1], dt.bfloat16)

            w_sb = {"wq": wq_sb, "wk": wk_sb, "wv": wv_sb}

            def emit_wq(q, eng):
                lo = WQUARTER * q
                for nm in ("wk", "wv", "wq"):
                    eng.dma_start(
                        out=w_sb[nm][:, lo:lo + WQUARTER, :], in_=w_d[(nm, q)][:]
                    )

            emit_wq(0, nc.sync)
            make_identity(nc, ident[:])
            make_upper_triangular(nc, tri[:], val=1.0, diag=True)
            nc.vector.memset(ones_sb[:], 1.0)

            # ---- phase 1a: own-half x load + K/V/Q-own accumulation ----
            ps_k = pbank("A", name="psk")
            ps_v = pbank("B", name="psv")
            ps_qo = pbank("C", name="psqo")
            for g, (cts, ring) in enumerate(XSCHED):
                eng = nc.scalar if ring == "scalar" else nc.sync
                if g in WQ_BEFORE:
                    q, wring = WQ_BEFORE[g]
                    emit_wq(q, nc.scalar if wring == "scalar" else nc.sync)
                c_lo, w = cts[0], len(cts)
                eng.dma_start(
                    out=xg_sb[g][:],
                    in_=xkvT[P * c_lo:P * (c_lo + w), 0:1024].rearrange(
                        "(j p) t -> p j t", p=P
                    ),
                )
                for jj, j in enumerate(cts):
                    st, sp = j == 0, j == NCT - 1
                    for n in range(2):
                        nc.tensor.matmul(
                            ps_k[:, 512 * n:512 * (n + 1)],
                            lhsT=wk_sb[:, j, :],
                            rhs=xg_sb[g][:, jj, 512 * n:512 * (n + 1)],
                            start=st, stop=sp,
                        )
                    for n in range(2):
                        nc.tensor.matmul(
                            ps_v[:, 512 * n:512 * (n + 1)],
                            lhsT=wv_sb[:, j, :],
                            rhs=xg_sb[g][:, jj, 512 * n:512 * (n + 1)],
                            start=st, stop=sp,
                        )
                    for n in range(2):
                        nc.tensor.matmul(
                            ps_qo[:, 512 * n:512 * (n + 1)],
                            lhsT=wq_sb[:, j, :],
                            rhs=xg_sb[g][:, jj, 512 * n:512 * (n + 1)],
                            start=st, stop=sp,
                        )

            # sib-half x streams in the background (needed from attention
            # half0 iter 0 onward); odd scalar rides along
            for g, (ring, _) in enumerate(SIBSCHED):
                eng = nc.scalar if ring == "scalar" else nc.sync
                eng.dma_start(
                    out=xs_sb[g][:],
                    in_=xkvT[
                        P * WQUARTER * g:P * WQUARTER * (g + 1), 1024:2048
                    ].rearrange("(j p) t -> p j t", p=P),
                )
            nc.sync.dma_start(out=odd_sb[:], in_=odd_d[:])

            # preload the ACT exp table (after the DMA issues so the table
            # load does not delay chunk 0 on the scalar queue)
            warm_sb = persist.tile([P, 1], dt.float32)
            nc.scalar.activation(
                warm_sb[:], ones_sb[:], mybir.ActivationFunctionType.Exp
            )

            # ---- phase boundary: PSUM -> SBUF ----
            # DVE: k then q-own (gates S j=0); ACT: vt (gpsimd has no PSUM
            # port); narrow first k copy so S(0) starts early
            nc.scalar.activation(
                vt_sb[:, 0:512], ps_v[:, 0:512],
                mybir.ActivationFunctionType.Copy,
            )
            nc.vector.tensor_copy(k_sb[:, 0:256], ps_k[:, 0:256])
            nc.vector.tensor_copy(q_sb[:, 0:512], ps_qo[:, 0:512])
            nc.vector.tensor_copy(q_sb[:, 512:1024], ps_qo[:, 512:1024])
            nc.scalar.activation(
                vt_sb[:, 512:1024], ps_v[:, 512:1024],
                mybir.ActivationFunctionType.Copy,
            )
            nc.vector.tensor_copy(k_sb[:, 256:512], ps_k[:, 256:512])
            nc.vector.tensor_copy(k_sb[:, 512:1024], ps_k[:, 512:1024])

            # ---- V tiles via PE transpose (tags B/D ping-pong, keeping A
            # and C free for half0's S rotation) ----
            for kt in range(NKT):
                ps_t = pbank(
                    "BD"[kt % 2], shape=(P, P), dtype=dt.bfloat16, name="pst"
                )
                nc.tensor.transpose(
                    ps_t[:], vt_sb[:, kt * P:(kt + 1) * P], ident[:]
                )
                nc.vector.tensor_copy(v_sb[:, kt, :], ps_t[:])

            # ---- attention ----
            ps_qs = pbank("D", name="psqs")   # deferred Q-sib accumulator

            def qsib_mm(it):
                # 8 matmuls per half0 iteration it (c-tiles 4*it .. 4*it+3)
                for jj in range(WQUARTER):
                    j = WQUARTER * it + jj
                    st, sp = j == 0, j == NCT - 1
                    for n in range(2):
                        nc.tensor.matmul(
                            ps_qs[:, 512 * n:512 * (n + 1)],
                            lhsT=wq_sb[:, j, :],
                            rhs=xs_sb[it][:, jj, 512 * n:512 * (n + 1)],
                            start=st, stop=sp,
                        )

            def den_mm(tag, half, n, den_sb):
                ps_d = pbank(tag, shape=(1, 512), name="psd")
                nc.tensor.matmul(
                    ps_d[:],
                    lhsT=ones_sb[:],
                    rhs=asum[half][:, 512 * n:512 * (n + 1)],
                    start=True, stop=True,
                )
                nc.vector.tensor_copy(den_sb[:, 512 * n:512 * (n + 1)], ps_d[:])

            def attention_half(half, s_tags, o_tag, den_tags):
                base = 1024 * half
                ps_o = pbank(o_tag, name=f"pso{half}")
                den_sb = outp.tile([1, 1024], dt.float32, name="den_sb")
                a_tiles = [None] * NKT

                def emit_S(j):
                    # S matmuls + one wide exp for key tile j
                    c0 = P * j
                    ps_s = pbank(s_tags[j % len(s_tags)], name="pss")
                    a_sb = ephem.tile([P, 1024], dt.bfloat16, name="a_sb")
                    a_tiles[j] = a_sb
                    if c0 < 512:
                        nc.tensor.matmul(
                            ps_s[:, c0:512],
                            lhsT=k_sb[:, c0:c0 + P],
                            rhs=q_sb[:, base + c0:base + 512],
                            start=True, stop=True,
                        )
                    lo2 = max(c0, 512)
                    nc.tensor.matmul(
                        ps_s[:, lo2:1024],
                        lhsT=k_sb[:, c0:c0 + P],
                        rhs=q_sb[:, base + lo2:base + 1024],
                        start=True, stop=True,
                    )
                    nc.scalar.activation(
                        a_sb[:, c0:1024], ps_s[:, c0:1024],
                        mybir.ActivationFunctionType.Exp,
                        scale=SCALE,
                    )
                    # diag block mask (DVE, ahead of the asum adds)
                    if half == 0:
                        nc.vector.tensor_mul(
                            a_sb[:, c0:c0 + P], a_sb[:, c0:c0 + P], tri[:]
                        )
                    else:
                        nc.vector.tensor_scalar_mul(
                            a_sb[:, c0:c0 + P], a_sb[:, c0:c0 + P], odd_sb[:]
                        )

                def asum_update(j):
                    # deferred one iteration so the DVE never delays a mask
                    c0 = P * j
                    if j == 0:
                        nc.vector.tensor_copy(asum[half][:, 0:1024], a_tiles[0][:])
                    else:
                        nc.vector.tensor_add(
                            asum[half][:, c0:1024],
                            asum[half][:, c0:1024],
                            a_tiles[j][:, c0:1024],
                        )

                ot_sb = outp.tile([P, 1024], dt.bfloat16, name="ot_sb")
                emit_S(0)
                emit_S(1)
                for j in range(NKT):
                    # S first (keeps ACT fed), then Q-sib filler so the PE
                    # has dependency-free work while exp(j) completes, then O
                    if j + 2 < NKT:
                        emit_S(j + 2)
                    if half == 0 and j < 4:
                        qsib_mm(j)
                    c0 = P * j
                    lo2 = max(c0, 512)
                    a_sb = a_tiles[j]
                    # O accumulation (cols 0:512 from j<=3; 512:1024 all j)
                    if c0 < 512:
                        nc.tensor.matmul(
                            ps_o[:, c0:512],
                            lhsT=v_sb[:, j, :],
                            rhs=a_sb[:, c0:512],
                            start=j == 0, stop=j == 3,
                        )
                    nc.tensor.matmul(
                        ps_o[:, lo2:1024],
                        lhsT=v_sb[:, j, :],
                        rhs=a_sb[:, lo2:1024],
                        start=j == 0, stop=j == NKT - 1,
                    )
                    if half == 0 and j == 4:
                        # Q-sib complete: copy for half1
                        nc.vector.tensor_copy(
                            q_sb[:, 1024:1536], ps_qs[:, 0:512]
                        )
                        nc.vector.tensor_copy(
                            q_sb[:, 1536:2048], ps_qs[:, 512:1024]
                        )
                    if j > 0:
                        asum_update(j - 1)
                    if j == 5:
                        # O cols [0,512) final since j==3: ship them early
                        nc.vector.tensor_copy(ot_sb[:, 0:512], ps_o[:, 0:512])
                        nc.sync.dma_start(
                            out=ot_d[:, base:base + 512], in_=ot_sb[:, 0:512]
                        )

                asum_update(NKT - 1)
                den_mm(den_tags[0], half, 0, den_sb)
                den_mm(den_tags[1], half, 1, den_sb)

                nc.vector.tensor_copy(ot_sb[:, 512:1024], ps_o[:, 512:1024])
                nc.sync.dma_start(
                    out=ot_d[:, base + 512:base + 1024], in_=ot_sb[:, 512:1024]
                )
                nc.sync.dma_start(out=den_d[:, base:base + 1024], in_=den_sb[:])

            attention_half(0, s_tags=["A", "C"], o_tag="B", den_tags=("A", "A"))
            attention_half(1, s_tags=["D", "A", "B"], o_tag="C",
                           den_tags=("D", "A"))

    nc.compile()
    return nc


def _core_perm(core):
    par = core % 2
    own = [2 * m + par for m in range(NKT)]
    sib = [2 * m + 1 - par for m in range(NKT)]
    return own + sib


def _prep_inputs(x, Wq, Wk, Wv):
    """Build the 8 per-core input maps."""
    def wshape(w):
        # [C, H] -> [128, NCT, H]: w_r[p, j, h] = w[j*128 + p, h]
        return np.ascontiguousarray(
            w.astype(BF16).reshape(NCT, P, H).transpose(1, 0, 2)
        )

    wmap = {}
    for nm, w in (("wq", Wq), ("wk", Wk), ("wv", Wv)):
        wb = wshape(w)
        for q in range(NCT // WQUARTER):
            wmap[f"{nm}q{q}"] = np.ascontiguousarray(
                wb[:, WQUARTER * q:WQUARTER * (q + 1), :]
            )
    x_bf = x.astype(BF16)

    in_maps = []
    for core in range(N_CORES):
        b, par = core // 2, core % 2
        cols = np.concatenate(
            [np.arange(P * t, P * t + P) for t in _core_perm(core)]
        )
        xT = np.ascontiguousarray(x_bf[b].T[:, cols])
        odd = np.full((P, 1), 1.0 - par, np.float32)
        in_maps.append({
            "xkvT": xT,
            **wmap,
            "odd": np.ascontiguousarray(odd),
        })
    return in_maps


def _assemble(results):
    num = np.zeros((B, T, H), np.float32)
    den = np.zeros((B, T, 1), np.float32)
    for core in range(N_CORES):
        b = core // 2
        r = results[core]
        oT = r["ot"].astype(np.float32).T   # [2048, H]
        dT = r["den"].T                     # [2048, 1]
        for i, g in enumerate(_core_perm(core)):
            num[b, P * g:P * (g + 1)] += oT[P * i:P * (i + 1)]
            den[b, P * g:P * (g + 1)] += dT[P * i:P * (i + 1)]
    return num / den


def _run(inputs, trace=False, **spmd_kwargs):
    from concourse.bass_utils import run_bass_kernel_spmd

    if "nc" not in _cache:
        _cache["nc"] = _build()
    nc = _cache["nc"]
    in_maps = _prep_inputs(
        np.asarray(inputs["x"], np.float32),
        np.asarray(inputs["Wq"], np.float32),
        np.asarray(inputs["Wk"], np.float32),
        np.asarray(inputs["Wv"], np.float32),
    )
    res = run_bass_kernel_spmd(
        nc, in_maps, list(range(N_CORES)), trace=trace, **spmd_kwargs
    )
    return _assemble(res.results), res


def kernel(x, Wq, Wk, Wv):
    out, _ = _run({"x": x, "Wq": Wq, "Wk": Wk, "Wv": Wv})
    return out


# revision 45
# speedup vs baseline: 1.1984x; 1.1984x over previous
"""Single-head causal attention (B=4, T=2048, C=2048, H=128) on 8 TRN2 cores.

Partial-attention sharding, no inter-core communication: 2 cores per batch.
Core (2b + par) owns the 8 key tiles {128*(2m+par)} of batch b and computes
  - K^T, V^T for its 1024 own key positions only,
  - Q^T for ALL 2048 query positions of the batch,
  - partial attention:   ot = sum_{own k} exp(s) * v,  den = sum_{own k} exp(s)
over every query. The host adds the two cores' partials per batch and divides
(softmax sums commute; the max-shift is skipped since |s| < ~6).

Per-core x.T is column-permuted to [own tiles | sib tiles]; with that order
the kernel is SPMD-identical:
  - K/V project from cols [0, 1024), Q from all cols,
  - attention key tile j covers query cols [128j, 1024) of BOTH halves:
    own-half diag block gets the triangular mask, sib-half diag block a
    per-core scalar (par=0 -> 1.0, par=1 -> 0.0).

Schedule (from trace analysis; phase 1 used to be PE-bound and serial):
  phase 1a projects K/V/Q-OWN only (6 matmuls per c-tile) and needs just
  the own-half columns of x (4.2MB instead of 8.4MB), so it ends ~9us
  earlier; the Q-SIB projection (32 matmuls) is deferred into attention
  half0's ACT-bound gaps, with the sib-half of x streaming lazily in the
  background. Attention half1 then uses the freshly copied Q-sib.

  PSUM as four 2-bank [128,1024] fp32 tags A-D:
    1a: K->A, V->B, Qown->C.  Transposes on B/D.  Qsib accumulates in D
    during half0 iters 0-3 (copied out at iter 4).
    half0: O on B, S rotates [A, C], dens on A.
    half1: O on C, S rotates [D, A, B], dens on D, A.
  den = one ones-matmul per 512 cols over a bf16 A_sum accumulated on the
  DVE. S is emitted 2 key tiles ahead of O; masks sit ahead of the
  deferred A_sum adds on the DVE queue. exp is one wide ACT instruction
  per key tile. Outputs are bf16; O cols [0,512) ship at iter 4.
"""

import numpy as np
import ml_dtypes

B, T, C, H = 4, 2048, 2048, 128
P = 128                 # tile edge
NCT = C // P            # 16 contraction c-tiles
NKT = 8                 # own key tiles per core
NQ = 2048               # query cols per core (own 1024 | sib 1024)
N_CORES = 8
SCALE = float(H) ** -0.5
BF16 = ml_dtypes.bfloat16

WQUARTER = 4            # weight c-tiles per DMA chunk (4 chunks per tensor)
# own-half x chunks: (c-tile list, ring); weight quarter q ships just
# before the x chunk at index WQ_BEFORE[g] (quarter 0 before everything)
XSCHED = [
    ([0], "scalar"), ([1], "sync"), ([2, 3], "scalar"), ([4, 5], "sync"),
    ([6, 7], "scalar"), ([8, 9], "sync"), ([10, 11], "scalar"),
    ([12, 13], "sync"), ([14], "scalar"), ([15], "scalar"),
]
WQ_BEFORE = {3: (1, "sync"), 5: (2, "sync"), 7: (3, "sync")}
# sib-half x chunks (4 c-tiles each), needed only by the deferred Q-sib
# matmuls in attention half0
SIBSCHED = [("scalar", 0), ("sync", 1), ("scalar", 2), ("sync", 3)]

_cache = {}


def _build():
    import concourse.bass as bass
    import concourse.mybir as mybir
    import concourse.tile as tile
    from concourse import bacc
    from concourse.masks import make_identity, make_upper_triangular

    dt = mybir.dt
    nc = bacc.Bacc(
        "TRN2",
        target_bir_lowering=False,
        debug=False,
        enable_asserts=False,
        num_devices=N_CORES,
    )

    xkvT = nc.dram_tensor("xkvT", [C, T], dt.bfloat16, kind="ExternalInput").ap()
    w_d = {
        (nm, q): nc.dram_tensor(
            f"{nm}q{q}", [P, WQUARTER, H], dt.bfloat16, kind="ExternalInput"
        ).ap()
        for nm in ("wq", "wk", "wv")
        for q in range(NCT // WQUARTER)
    }
    # sib-half diag block allowed: 1.0 on par=0 cores, 0.0 on par=1 cores
    odd_d = nc.dram_tensor("odd", [P, 1], dt.float32, kind="ExternalInput").ap()
    ot_d = nc.dram_tensor("ot", [H, NQ], dt.bfloat16, kind="ExternalOutput").ap()
    den_d = nc.dram_tensor("den", [1, NQ], dt.float32, kind="ExternalOutput").ap()

    with tile.TileContext(nc) as tc:
        with (
            tc.tile_pool(name="persist", bufs=1) as persist,
            tc.tile_pool(name="ephem", bufs=8) as ephem,
            tc.tile_pool(name="outp", bufs=2) as outp,
            tc.tile_pool(name="psum", bufs=1, space="PSUM") as psum,
        ):
            # PSUM = four 2-bank [128,1024] fp32 tiles, tags A..D
            def pbank(tag, shape=(P, 1024), dtype=dt.float32, name=None):
                return psum.tile(
                    list(shape), dtype, tag=f"pp{tag}", name=name or f"pp{tag}"
                )

            wq_sb = persist.tile([P, NCT, H], dt.bfloat16)
            wk_sb = persist.tile([P, NCT, H], dt.bfloat16)
            wv_sb = persist.tile([P, NCT, H], dt.bfloat16)
            odd_sb = persist.tile([P, 1], dt.float32)
            xg_sb = [
                persist.tile([P, len(cts), 1024], dt.bfloat16, name=f"xg{g}")
                for g, (cts, _) in enumerate(XSCHED)
            ]
            xs_sb = [
                persist.tile([P, WQUARTER, 1024], dt.bfloat16, name=f"xs{g}")
                for g in range(len(SIBSCHED))
            ]
            k_sb = persist.tile([P, P * NKT], dt.bfloat16)   # K^T own [h, 1024]
            vt_sb = persist.tile([P, P * NKT], dt.bfloat16)  # V^T own [h, 1024]
            v_sb = persist.tile([P, NKT, H], dt.bfloat16)    # V tiles [k, h]
            q_sb = persist.tile([P, NQ], dt.bfloat16)        # Q^T [h, 2048]
            # A_sum in bf16: half the DVE add cost, den matmul reads it
            # directly (den rel err ~0.5%, well inside the 2e-2 gate)
            asum = [
                persist.tile([P, 1024], dt.bfloat16, name=f"asum{h}") for h in (0, 1)
            ]
            ident = persist.tile([P, P], dt.bfloat16)
            tri = persist.tile([P, P], dt.bfloat16)          # 1 where k <= q
            ones_sb = persist.tile([P, 1], dt.bfloat16)

            w_sb = {"wq": wq_sb, "wk": wk_sb, "wv": wv_sb}

            def emit_wq(q, eng):
                lo = WQUARTER * q
                for nm in ("wk", "wv", "wq"):
                    eng.dma_start(
                        out=w_sb[nm][:, lo:lo + WQUARTER, :], in_=w_d[(nm, q)][:]
                    )

            emit_wq(0, nc.sync)
            make_identity(nc, ident[:])
            make_upper_triangular(nc, tri[:], val=1.0, diag=True)
            nc.vector.memset(ones_sb[:], 1.0)

            # ---- phase 1a: own-half x load + K/V/Q-own accumulation ----
            ps_k = pbank("A", name="psk")
            ps_v = pbank("B", name="psv")
            ps_qo = pbank("C", name="psqo")
            for g, (cts, ring) in enumerate(XSCHED):
                eng = nc.scalar if ring == "scalar" else nc.sync
                if g in WQ_BEFORE:
                    q, wring = WQ_BEFORE[g]
                    emit_wq(q, nc.scalar if wring == "scalar" else nc.sync)
                c_lo, w = cts[0], len(cts)
                eng.dma_start(
                    out=xg_sb[g][:],
                    in_=xkvT[P * c_lo:P * (c_lo + w), 0:1024].rearrange(
                        "(j p) t -> p j t", p=P
                    ),
                )
                for jj, j in enumerate(cts):
                    st, sp = j == 0, j == NCT - 1
                    for n in range(2):
                        nc.tensor.matmul(
                            ps_k[:, 512 * n:512 * (n + 1)],
                            lhsT=wk_sb[:, j, :],
                            rhs=xg_sb[g][:, jj, 512 * n:512 * (n + 1)],
                            start=st, stop=sp,
                        )
                    for n in range(2):
                        nc.tensor.matmul(
                            ps_v[:, 512 * n:512 * (n + 1)],
                            lhsT=wv_sb[:, j, :],
                            rhs=xg_sb[g][:, jj, 512 * n:512 * (n + 1)],
                            start=st, stop=sp,
                        )
                    for n in range(2):
                        nc.tensor.matmul(
                            ps_qo[:, 512 * n:512 * (n + 1)],
                            lhsT=wq_sb[:, j, :],
                            rhs=xg_sb[g][:, jj, 512 * n:512 * (n + 1)],
                            start=st, stop=sp,
                        )

            # sib-half x streams in the background (needed from attention
            # half0 iter 0 onward); odd scalar rides along
            for g, (ring, _) in enumerate(SIBSCHED):
                eng = nc.scalar if ring == "scalar" else nc.sync
                eng.dma_start(
                    out=xs_sb[g][:],
                    in_=xkvT[
                        P * WQUARTER * g:P * WQUARTER * (g + 1), 1024:2048
                    ].rearrange("(j p) t -> p j t", p=P),
                )
            nc.sync.dma_start(out=odd_sb[:], in_=odd_d[:])

            # preload the ACT exp table (after the DMA issues so the table
            # load does not delay chunk 0 on the scalar queue)
            warm_sb = persist.tile([P, 1], dt.float32)
            nc.scalar.activation(
                warm_sb[:], ones_sb[:], mybir.ActivationFunctionType.Exp
            )

            # ---- phase boundary: PSUM -> SBUF ----
            # DVE: k then q-own (gates S j=0); ACT: vt (gpsimd has no PSUM
            # port); narrow first k copy so S(0) starts early
            nc.scalar.activation(
                vt_sb[:, 0:512], ps_v[:, 0:512],
                mybir.ActivationFunctionType.Copy,
            )
            nc.vector.tensor_copy(k_sb[:, 0:256], ps_k[:, 0:256])
            nc.vector.tensor_copy(q_sb[:, 0:512], ps_qo[:, 0:512])
            nc.vector.tensor_copy(q_sb[:, 512:1024], ps_qo[:, 512:1024])
            nc.scalar.activation(
                vt_sb[:, 512:1024], ps_v[:, 512:1024],
                mybir.ActivationFunctionType.Copy,
            )
            nc.vector.tensor_copy(k_sb[:, 256:512], ps_k[:, 256:512])
            nc.vector.tensor_copy(k_sb[:, 512:1024], ps_k[:, 512:1024])

            # ---- V tiles via PE transpose (tags B/D ping-pong, keeping A
            # and C free for half0's S rotation) ----
            for kt in range(NKT):
                ps_t = pbank(
                    "BD"[kt % 2], shape=(P, P), dtype=dt.bfloat16, name="pst"
                )
                nc.tensor.transpose(
                    ps_t[:], vt_sb[:, kt * P:(kt + 1) * P], ident[:]
                )
                nc.vector.tensor_copy(v_sb[:, kt, :], ps_t[:])

            # ---- attention ----
            ps_qs = pbank("D", name="psqs")   # deferred Q-sib accumulator

            def qsib_mm(it):
                # 8 matmuls per half0 iteration it (c-tiles 4*it .. 4*it+3)
                for jj in range(WQUARTER):
                    j = WQUARTER * it + jj
                    st, sp = j == 0, j == NCT - 1
                    for n in range(2):
                        nc.tensor.matmul(
                            ps_qs[:, 512 * n:512 * (n + 1)],
                            lhsT=wq_sb[:, j, :],
                            rhs=xs_sb[it][:, jj, 512 * n:512 * (n + 1)],
                            start=st, stop=sp,
                        )

            def den_mm(tag, half, n, den_sb):
                ps_d = pbank(tag, shape=(1, 512), name="psd")
                nc.tensor.matmul(
                    ps_d[:],
                    lhsT=ones_sb[:],
                    rhs=asum[half][:, 512 * n:512 * (n + 1)],
                    start=True, stop=True,
                )
                nc.vector.tensor_copy(den_sb[:, 512 * n:512 * (n + 1)], ps_d[:])

            def attention_half(half, s_tags, o_tag, den_tags, lookahead):
                base = 1024 * half
                ps_o = pbank(o_tag, name=f"pso{half}")
                den_sb = outp.tile([1, 1024], dt.float32, name="den_sb")
                a_tiles = [None] * NKT

                def emit_S(j):
                    # S matmuls + one wide exp for key tile j
                    c0 = P * j
                    ps_s = pbank(s_tags[j % len(s_tags)], name="pss")
                    a_sb = ephem.tile([P, 1024], dt.bfloat16, name="a_sb")
                    a_tiles[j] = a_sb
                    if c0 < 512:
                        nc.tensor.matmul(
                            ps_s[:, c0:512],
                            lhsT=k_sb[:, c0:c0 + P],
                            rhs=q_sb[:, base + c0:base + 512],
                            start=True, stop=True,
                        )
                    lo2 = max(c0, 512)
                    nc.tensor.matmul(
                        ps_s[:, lo2:1024],
                        lhsT=k_sb[:, c0:c0 + P],
                        rhs=q_sb[:, base + lo2:base + 1024],
                        start=True, stop=True,
                    )
                    nc.scalar.activation(
                        a_sb[:, c0:1024], ps_s[:, c0:1024],
                        mybir.ActivationFunctionType.Exp,
                        scale=SCALE,
                    )
                    # diag block mask (DVE, ahead of the asum adds)
                    if half == 0:
                        nc.vector.tensor_mul(
                            a_sb[:, c0:c0 + P], a_sb[:, c0:c0 + P], tri[:]
                        )
                    else:
                        nc.vector.tensor_scalar_mul(
                            a_sb[:, c0:c0 + P], a_sb[:, c0:c0 + P], odd_sb[:]
                        )

                def asum_update(j):
                    # deferred one iteration so the DVE never delays a mask
                    c0 = P * j
                    if j == 0:
                        nc.vector.tensor_copy(asum[half][:, 0:1024], a_tiles[0][:])
                    else:
                        nc.vector.tensor_add(
                            asum[half][:, c0:1024],
                            asum[half][:, c0:1024],
                            a_tiles[j][:, c0:1024],
                        )

                ot_sb = outp.tile([P, 1024], dt.bfloat16, name="ot_sb")
                for l in range(lookahead):
                    emit_S(l)
                for j in range(NKT):
                    # with a 2-tag rotation, +1 lookahead means S(j+1)
                    # tag-waits exp(j-1) (already done) instead of exp(j)
                    if j + lookahead < NKT:
                        emit_S(j + lookahead)
                    c0 = P * j
                    lo2 = max(c0, 512)
                    a_sb = a_tiles[j]
                    # O accumulation (cols 0:512 from j<=3; 512:1024 all j)
                    if c0 < 512:
                        nc.tensor.matmul(
                            ps_o[:, c0:512],
                            lhsT=v_sb[:, j, :],
                            rhs=a_sb[:, c0:512],
                            start=j == 0, stop=j == 3,
                        )
                    nc.tensor.matmul(
                        ps_o[:, lo2:1024],
                        lhsT=v_sb[:, j, :],
                        rhs=a_sb[:, lo2:1024],
                        start=j == 0, stop=j == NKT - 1,
                    )
                    if half == 0 and j < 4:
                        # deferred Q-sib projection as PE filler
                        qsib_mm(j)
                    if half == 0 and j == 4:
                        # Q-sib complete: copy for half1
                        nc.vector.tensor_copy(
                            q_sb[:, 1024:1536], ps_qs[:, 0:512]
                        )
                        nc.vector.tensor_copy(
                            q_sb[:, 1536:2048], ps_qs[:, 512:1024]
                        )
                    if j > 0:
                        asum_update(j - 1)
                    if j == 5:
                        # O cols [0,512) final since j==3: ship them early
                        nc.vector.tensor_copy(ot_sb[:, 0:512], ps_o[:, 0:512])
                        nc.sync.dma_start(
                            out=ot_d[:, base:base + 512], in_=ot_sb[:, 0:512]
                        )

                asum_update(NKT - 1)
                den_mm(den_tags[0], half, 0, den_sb)
                den_mm(den_tags[1], half, 1, den_sb)

                nc.vector.tensor_copy(ot_sb[:, 512:1024], ps_o[:, 512:1024])
                nc.sync.dma_start(
                    out=ot_d[:, base + 512:base + 1024], in_=ot_sb[:, 512:1024]
                )
                nc.sync.dma_start(out=den_d[:, base:base + 1024], in_=den_sb[:])

            attention_half(0, s_tags=["A", "C"], o_tag="B",
                           den_tags=("A", "A"), lookahead=1)
            attention_half(1, s_tags=["D", "A", "B"], o_tag="C",
                           den_tags=("D", "A"), lookahead=2)

    nc.compile()
    return nc


def _core_perm(core):
    par = core % 2
    own = [2 * m + par for m in range(NKT)]
    sib = [2 * m + 1 - par for m in range(NKT)]
    return own + sib


def _prep_inputs(x, Wq, Wk, Wv):
    """Build the 8 per-core input maps."""
    def wshape(w):
        # [C, H] -> [128, NCT, H]: w_r[p, j, h] = w[j*128 + p, h]
        return np.ascontiguousarray(
            w.astype(BF16).reshape(NCT, P, H).transpose(1, 0, 2)
        )

    wmap = {}
    for nm, w in (("wq", Wq), ("wk", Wk), ("wv", Wv)):
        wb = wshape(w)
        for q in range(NCT // WQUARTER):
            wmap[f"{nm}q{q}"] = np.ascontiguousarray(
                wb[:, WQUARTER * q:WQUARTER * (q + 1), :]
            )
    x_bf = x.astype(BF16)

    in_maps = []
    for core in range(N_CORES):
        b, par = core // 2, core % 2
        cols = np.concatenate(
            [np.arange(P * t, P * t + P) for t in _core_perm(core)]
        )
        xT = np.ascontiguousarray(x_bf[b].T[:, cols])
        odd = np.full((P, 1), 1.0 - par, np.float32)
        in_maps.append({
            "xkvT": xT,
            **wmap,
            "odd": np.ascontiguousarray(odd),
        })
    return in_maps


def _assemble(results):
    num = np.zeros((B, T, H), np.float32)
    den = np.zeros((B, T, 1), np.float32)
    for core in range(N_CORES):
        b = core // 2
        r = results[core]
        oT = r["ot"].astype(np.float32).T   # [2048, H]
        dT = r["den"].T                     # [2048, 1]
        for i, g in enumerate(_core_perm(core)):
            num[b, P * g:P * (g + 1)] += oT[P * i:P * (i + 1)]
            den[b, P * g:P * (g + 1)] += dT[P * i:P * (i + 1)]
    return num / den


def _run(inputs, trace=False, **spmd_kwargs):
    from concourse.bass_utils import run_bass_kernel_spmd

    if "nc" not in _cache:
        _cache["nc"] = _build()
    nc = _cache["nc"]
    in_maps = _prep_inputs(
        np.asarray(inputs["x"], np.float32),
        np.asarray(inputs["Wq"], np.float32),
        np.asarray(inputs["Wk"], np.float32),
        np.asarray(inputs["Wv"], np.float32),
    )
    res = run_bass_kernel_spmd(
        nc, in_maps, list(range(N_CORES)), trace=trace, **spmd_kwargs
    )
    return _assemble(res.results), res


def kernel(x, Wq, Wk, Wv):
    out, _ = _run({"x": x, "Wq": Wq, "Wk": Wk, "Wv": Wv})
    return out


# revision 46
# speedup vs baseline: 1.2002x; 1.0015x over previous
"""Single-head causal attention (B=4, T=2048, C=2048, H=128) on 8 TRN2 cores.

Partial-attention sharding, no inter-core communication: 2 cores per batch.
Core (2b + par) owns the 8 key tiles {128*(2m+par)} of batch b and computes
  - K^T, V^T for its 1024 own key positions only,
  - Q^T for ALL 2048 query positions of the batch,
  - partial attention:   ot = sum_{own k} exp(s) * v,  den = sum_{own k} exp(s)
over every query. The host adds the two cores' partials per batch and divides
(softmax sums commute; the max-shift is skipped since |s| < ~6).

Per-core x.T is column-permuted to [own tiles | sib tiles]; with that order
the kernel is SPMD-identical:
  - K/V project from cols [0, 1024), Q from all cols,
  - attention key tile j covers query cols [128j, 1024) of BOTH halves:
    own-half diag block gets the triangular mask, sib-half diag block a
    per-core scalar (par=0 -> 1.0, par=1 -> 0.0).

Schedule (from trace analysis; phase 1 used to be PE-bound and serial):
  phase 1a projects K/V/Q-OWN only (6 matmuls per c-tile) and needs just
  the own-half columns of x (4.2MB instead of 8.4MB), so it ends ~9us
  earlier; the Q-SIB projection (32 matmuls) is deferred into attention
  half0's ACT-bound gaps, with the sib-half of x streaming lazily in the
  background. Attention half1 then uses the freshly copied Q-sib.

  PSUM as four 2-bank [128,1024] fp32 tags A-D:
    1a: K->A, V->B, Qown->C.  Transposes on B/D.  Qsib accumulates in D
    during half0 iters 0-3 (copied out at iter 4).
    half0: O on B, S rotates [A, C], dens on A.
    half1: O on C, S rotates [D, A, B], dens on D, A.
  den = one ones-matmul per 512 cols over a bf16 A_sum accumulated on the
  DVE. S is emitted 2 key tiles ahead of O; masks sit ahead of the
  deferred A_sum adds on the DVE queue. exp is one wide ACT instruction
  per key tile. Outputs are bf16; O cols [0,512) ship at iter 4.
"""

import numpy as np
import ml_dtypes

B, T, C, H = 4, 2048, 2048, 128
P = 128                 # tile edge
NCT = C // P            # 16 contraction c-tiles
NKT = 8                 # own key tiles per core
NQ = 2048               # query cols per core (own 1024 | sib 1024)
N_CORES = 8
SCALE = float(H) ** -0.5
BF16 = ml_dtypes.bfloat16

WQUARTER = 4            # weight c-tiles per DMA chunk (4 chunks per tensor)
# own-half x chunks: (c-tile list, ring); weight quarter q ships just
# before the x chunk at index WQ_BEFORE[g] (quarter 0 before everything)
XSCHED = [
    ([0], "scalar"), ([1], "sync"), ([2], "scalar"), ([3], "sync"),
    ([4], "scalar"), ([5], "sync"), ([6, 7], "scalar"), ([8, 9], "sync"),
    ([10, 11], "scalar"), ([12, 13], "sync"), ([14, 15], "scalar"),
]
WQ_BEFORE = {3: (1, "sync"), 7: (2, "sync"), 9: (3, "sync")}
# sib-half x chunks (4 c-tiles each), needed only by the deferred Q-sib
# matmuls in attention half0
SIBSCHED = [("scalar", 0), ("sync", 1), ("scalar", 2), ("sync", 3)]

_cache = {}


def _build():
    import concourse.bass as bass
    import concourse.mybir as mybir
    import concourse.tile as tile
    from concourse import bacc
    from concourse.masks import make_identity, make_upper_triangular

    dt = mybir.dt
    nc = bacc.Bacc(
        "TRN2",
        target_bir_lowering=False,
        debug=False,
        enable_asserts=False,
        num_devices=N_CORES,
    )

    xkvT = nc.dram_tensor("xkvT", [C, T], dt.bfloat16, kind="ExternalInput").ap()
    w_d = {
        (nm, q): nc.dram_tensor(
            f"{nm}q{q}", [P, WQUARTER, H], dt.bfloat16, kind="ExternalInput"
        ).ap()
        for nm in ("wq", "wk", "wv")
        for q in range(NCT // WQUARTER)
    }
    # sib-half diag block allowed: 1.0 on par=0 cores, 0.0 on par=1 cores
    odd_d = nc.dram_tensor("odd", [P, 1], dt.float32, kind="ExternalInput").ap()
    ot_d = nc.dram_tensor("ot", [H, NQ], dt.bfloat16, kind="ExternalOutput").ap()
    den_d = nc.dram_tensor("den", [1, NQ], dt.float32, kind="ExternalOutput").ap()

    with tile.TileContext(nc) as tc:
        with (
            tc.tile_pool(name="persist", bufs=1) as persist,
            tc.tile_pool(name="ephem", bufs=8) as ephem,
            tc.tile_pool(name="outp", bufs=2) as outp,
            tc.tile_pool(name="psum", bufs=1, space="PSUM") as psum,
        ):
            # PSUM = four 2-bank [128,1024] fp32 tiles, tags A..D
            def pbank(tag, shape=(P, 1024), dtype=dt.float32, name=None):
                return psum.tile(
                    list(shape), dtype, tag=f"pp{tag}", name=name or f"pp{tag}"
                )

            wq_sb = persist.tile([P, NCT, H], dt.bfloat16)
            wk_sb = persist.tile([P, NCT, H], dt.bfloat16)
            wv_sb = persist.tile([P, NCT, H], dt.bfloat16)
            odd_sb = persist.tile([P, 1], dt.float32)
            xg_sb = [
                persist.tile([P, len(cts), 1024], dt.bfloat16, name=f"xg{g}")
                for g, (cts, _) in enumerate(XSCHED)
            ]
            xs_sb = [
                persist.tile([P, WQUARTER, 1024], dt.bfloat16, name=f"xs{g}")
                for g in range(len(SIBSCHED))
            ]
            k_sb = persist.tile([P, P * NKT], dt.bfloat16)   # K^T own [h, 1024]
            vt_sb = persist.tile([P, P * NKT], dt.bfloat16)  # V^T own [h, 1024]
            v_sb = persist.tile([P, NKT, H], dt.bfloat16)    # V tiles [k, h]
            q_sb = persist.tile([P, NQ], dt.bfloat16)        # Q^T [h, 2048]
            # A_sum in bf16: half the DVE add cost, den matmul reads it
            # directly (den rel err ~0.5%, well inside the 2e-2 gate)
            asum = [
                persist.tile([P, 1024], dt.bfloat16, name=f"asum{h}") for h in (0, 1)
            ]
            ident = persist.tile([P, P], dt.bfloat16)
            tri = persist.tile([P, P], dt.bfloat16)          # 1 where k <= q
            ones_sb = persist.tile([P, 1], dt.bfloat16)

            w_sb = {"wq": wq_sb, "wk": wk_sb, "wv": wv_sb}

            def emit_wq(q, eng):
                lo = WQUARTER * q
                for nm in ("wk", "wv", "wq"):
                    eng.dma_start(
                        out=w_sb[nm][:, lo:lo + WQUARTER, :], in_=w_d[(nm, q)][:]
                    )

            emit_wq(0, nc.sync)
            make_identity(nc, ident[:])
            make_upper_triangular(nc, tri[:], val=1.0, diag=True)
            nc.vector.memset(ones_sb[:], 1.0)

            # ---- phase 1a: own-half x load + K/V/Q-own accumulation ----
            ps_k = pbank("A", name="psk")
            ps_v = pbank("B", name="psv")
            ps_qo = pbank("C", name="psqo")
            for g, (cts, ring) in enumerate(XSCHED):
                eng = nc.scalar if ring == "scalar" else nc.sync
                if g in WQ_BEFORE:
                    q, wring = WQ_BEFORE[g]
                    emit_wq(q, nc.scalar if wring == "scalar" else nc.sync)
                c_lo, w = cts[0], len(cts)
                eng.dma_start(
                    out=xg_sb[g][:],
                    in_=xkvT[P * c_lo:P * (c_lo + w), 0:1024].rearrange(
                        "(j p) t -> p j t", p=P
                    ),
                )
                for jj, j in enumerate(cts):
                    st, sp = j == 0, j == NCT - 1
                    for n in range(2):
                        nc.tensor.matmul(
                            ps_k[:, 512 * n:512 * (n + 1)],
                            lhsT=wk_sb[:, j, :],
                            rhs=xg_sb[g][:, jj, 512 * n:512 * (n + 1)],
                            start=st, stop=sp,
                        )
                    for n in range(2):
                        nc.tensor.matmul(
                            ps_v[:, 512 * n:512 * (n + 1)],
                            lhsT=wv_sb[:, j, :],
                            rhs=xg_sb[g][:, jj, 512 * n:512 * (n + 1)],
                            start=st, stop=sp,
                        )
                    for n in range(2):
                        nc.tensor.matmul(
                            ps_qo[:, 512 * n:512 * (n + 1)],
                            lhsT=wq_sb[:, j, :],
                            rhs=xg_sb[g][:, jj, 512 * n:512 * (n + 1)],
                            start=st, stop=sp,
                        )

            # sib-half x streams in the background (needed from attention
            # half0 iter 0 onward); odd scalar rides along
            for g, (ring, _) in enumerate(SIBSCHED):
                eng = nc.scalar if ring == "scalar" else nc.sync
                eng.dma_start(
                    out=xs_sb[g][:],
                    in_=xkvT[
                        P * WQUARTER * g:P * WQUARTER * (g + 1), 1024:2048
                    ].rearrange("(j p) t -> p j t", p=P),
                )
            nc.sync.dma_start(out=odd_sb[:], in_=odd_d[:])

            # preload the ACT exp table (after the DMA issues so the table
            # load does not delay chunk 0 on the scalar queue)
            warm_sb = persist.tile([P, 1], dt.float32)
            nc.scalar.activation(
                warm_sb[:], ones_sb[:], mybir.ActivationFunctionType.Exp
            )

            # ---- phase boundary: PSUM -> SBUF ----
            # DVE: k then q-own (gates S j=0); ACT: vt (gpsimd has no PSUM
            # port); narrow first k copy so S(0) starts early
            nc.scalar.activation(
                vt_sb[:, 0:512], ps_v[:, 0:512],
                mybir.ActivationFunctionType.Copy,
            )
            nc.vector.tensor_copy(k_sb[:, 0:256], ps_k[:, 0:256])
            nc.vector.tensor_copy(q_sb[:, 0:512], ps_qo[:, 0:512])
            nc.vector.tensor_copy(q_sb[:, 512:1024], ps_qo[:, 512:1024])
            nc.scalar.activation(
                vt_sb[:, 512:1024], ps_v[:, 512:1024],
                mybir.ActivationFunctionType.Copy,
            )
            nc.vector.tensor_copy(k_sb[:, 256:512], ps_k[:, 256:512])
            nc.vector.tensor_copy(k_sb[:, 512:1024], ps_k[:, 512:1024])

            # ---- V tiles via PE transpose (tags B/D ping-pong, keeping A
            # and C free for half0's S rotation) ----
            for kt in range(NKT):
                ps_t = pbank(
                    "BD"[kt % 2], shape=(P, P), dtype=dt.bfloat16, name="pst"
                )
                nc.tensor.transpose(
                    ps_t[:], vt_sb[:, kt * P:(kt + 1) * P], ident[:]
                )
                nc.vector.tensor_copy(v_sb[:, kt, :], ps_t[:])

            # ---- attention ----
            ps_qs = pbank("D", name="psqs")   # deferred Q-sib accumulator

            def qsib_mm(it):
                # 8 matmuls per half0 iteration it (c-tiles 4*it .. 4*it+3)
                for jj in range(WQUARTER):
                    j = WQUARTER * it + jj
                    st, sp = j == 0, j == NCT - 1
                    for n in range(2):
                        nc.tensor.matmul(
                            ps_qs[:, 512 * n:512 * (n + 1)],
                            lhsT=wq_sb[:, j, :],
                            rhs=xs_sb[it][:, jj, 512 * n:512 * (n + 1)],
                            start=st, stop=sp,
                        )

            def den_mm(tag, half, n, den_sb):
                ps_d = pbank(tag, shape=(1, 512), name="psd")
                nc.tensor.matmul(
                    ps_d[:],
                    lhsT=ones_sb[:],
                    rhs=asum[half][:, 512 * n:512 * (n + 1)],
                    start=True, stop=True,
                )
                nc.vector.tensor_copy(den_sb[:, 512 * n:512 * (n + 1)], ps_d[:])

            def attention_half(half, s_tags, o_tag, den_tags, lookahead):
                base = 1024 * half
                ps_o = pbank(o_tag, name=f"pso{half}")
                den_sb = outp.tile([1, 1024], dt.float32, name="den_sb")
                a_tiles = [None] * NKT

                def emit_S(j):
                    # S matmuls + one wide exp for key tile j
                    c0 = P * j
                    ps_s = pbank(s_tags[j % len(s_tags)], name="pss")
                    a_sb = ephem.tile([P, 1024], dt.bfloat16, name="a_sb")
                    a_tiles[j] = a_sb
                    if c0 < 512:
                        nc.tensor.matmul(
                            ps_s[:, c0:512],
                            lhsT=k_sb[:, c0:c0 + P],
                            rhs=q_sb[:, base + c0:base + 512],
                            start=True, stop=True,
                        )
                    lo2 = max(c0, 512)
                    nc.tensor.matmul(
                        ps_s[:, lo2:1024],
                        lhsT=k_sb[:, c0:c0 + P],
                        rhs=q_sb[:, base + lo2:base + 1024],
                        start=True, stop=True,
                    )
                    nc.scalar.activation(
                        a_sb[:, c0:1024], ps_s[:, c0:1024],
                        mybir.ActivationFunctionType.Exp,
                        scale=SCALE,
                    )
                    # diag block mask (DVE, ahead of the asum adds)
                    if half == 0:
                        nc.vector.tensor_mul(
                            a_sb[:, c0:c0 + P], a_sb[:, c0:c0 + P], tri[:]
                        )
                    else:
                        nc.vector.tensor_scalar_mul(
                            a_sb[:, c0:c0 + P], a_sb[:, c0:c0 + P], odd_sb[:]
                        )

                def asum_update(j):
                    # deferred one iteration so the DVE never delays a mask
                    c0 = P * j
                    if j == 0:
                        nc.vector.tensor_copy(asum[half][:, 0:1024], a_tiles[0][:])
                    else:
                        nc.vector.tensor_add(
                            asum[half][:, c0:1024],
                            asum[half][:, c0:1024],
                            a_tiles[j][:, c0:1024],
                        )

                ot_sb = outp.tile([P, 1024], dt.bfloat16, name="ot_sb")
                for l in range(lookahead):
                    emit_S(l)
                for j in range(NKT):
                    # with a 2-tag rotation, +1 lookahead means S(j+1)
                    # tag-waits exp(j-1) (already done) instead of exp(j)
                    if j + lookahead < NKT:
                        emit_S(j + lookahead)
                    c0 = P * j
                    lo2 = max(c0, 512)
                    a_sb = a_tiles[j]
                    # O accumulation (cols 0:512 from j<=3; 512:1024 all j)
                    if c0 < 512:
                        nc.tensor.matmul(
                            ps_o[:, c0:512],
                            lhsT=v_sb[:, j, :],
                            rhs=a_sb[:, c0:512],
                            start=j == 0, stop=j == 3,
                        )
                    nc.tensor.matmul(
                        ps_o[:, lo2:1024],
                        lhsT=v_sb[:, j, :],
                        rhs=a_sb[:, lo2:1024],
                        start=j == 0, stop=j == NKT - 1,
                    )
                    if half == 0 and j < 4:
                        # deferred Q-sib projection as PE filler
                        qsib_mm(j)
                    if half == 0 and j == 4:
                        # Q-sib complete: copy for half1
                        nc.vector.tensor_copy(
                            q_sb[:, 1024:1536], ps_qs[:, 0:512]
                        )
                        nc.vector.tensor_copy(
                            q_sb[:, 1536:2048], ps_qs[:, 512:1024]
                        )
                    if j > 0:
                        asum_update(j - 1)
                    if j == 5:
                        # O cols [0,512) final since j==3: ship them early
                        nc.vector.tensor_copy(ot_sb[:, 0:512], ps_o[:, 0:512])
                        nc.sync.dma_start(
                            out=ot_d[:, base:base + 512], in_=ot_sb[:, 0:512]
                        )

                asum_update(NKT - 1)
                den_mm(den_tags[0], half, 0, den_sb)
                den_mm(den_tags[1], half, 1, den_sb)

                nc.vector.tensor_copy(ot_sb[:, 512:1024], ps_o[:, 512:1024])
                nc.sync.dma_start(
                    out=ot_d[:, base + 512:base + 1024], in_=ot_sb[:, 512:1024]
                )
                nc.sync.dma_start(out=den_d[:, base:base + 1024], in_=den_sb[:])

            attention_half(0, s_tags=["A", "C"], o_tag="B",
                           den_tags=("A", "A"), lookahead=1)
            attention_half(1, s_tags=["D", "A", "B"], o_tag="C",
                           den_tags=("D", "A"), lookahead=2)

    nc.compile()
    return nc


def _core_perm(core):
    par = core % 2
    own = [2 * m + par for m in range(NKT)]
    sib = [2 * m + 1 - par for m in range(NKT)]
    return own + sib


def _prep_inputs(x, Wq, Wk, Wv):
    """Build the 8 per-core input maps."""
    def wshape(w):
        # [C, H] -> [128, NCT, H]: w_r[p, j, h] = w[j*128 + p, h]
        return np.ascontiguousarray(
            w.astype(BF16).reshape(NCT, P, H).transpose(1, 0, 2)
        )

    wmap = {}
    for nm, w in (("wq", Wq), ("wk", Wk), ("wv", Wv)):
        wb = wshape(w)
        for q in range(NCT // WQUARTER):
            wmap[f"{nm}q{q}"] = np.ascontiguousarray(
                wb[:, WQUARTER * q:WQUARTER * (q + 1), :]
            )
    x_bf = x.astype(BF16)

    in_maps = []
    for core in range(N_CORES):
        b, par = core // 2, core % 2
        cols = np.concatenate(
            [np.arange(P * t, P * t + P) for t in _core_perm(core)]
        )
        xT = np.ascontiguousarray(x_bf[b].T[:, cols])
        odd = np.full((P, 1), 1.0 - par, np.float32)
        in_maps.append({
            "xkvT": xT,
            **wmap,
            "odd": np.ascontiguousarray(odd),
        })
    return in_maps


def _assemble(results):
    num = np.zeros((B, T, H), np.float32)
    den = np.zeros((B, T, 1), np.float32)
    for core in range(N_CORES):
        b = core // 2
        r = results[core]
        oT = r["ot"].astype(np.float32).T   # [2048, H]
        dT = r["den"].T                     # [2048, 1]
        for i, g in enumerate(_core_perm(core)):
            num[b, P * g:P * (g + 1)] += oT[P * i:P * (i + 1)]
            den[b, P * g:P * (g + 1)] += dT[P * i:P * (i + 1)]
    return num / den


def _run(inputs, trace=False, **spmd_kwargs):
    from concourse.bass_utils import run_bass_kernel_spmd

    if "nc" not in _cache:
        _cache["nc"] = _build()
    nc = _cache["nc"]
    in_maps = _prep_inputs(
        np.asarray(inputs["x"], np.float32),
        np.asarray(inputs["Wq"], np.float32),
        np.asarray(inputs["Wk"], np.float32),
        np.asarray(inputs["Wv"], np.float32),
    )
    res = run_bass_kernel_spmd(
        nc, in_maps, list(range(N_CORES)), trace=trace, **spmd_kwargs
    )
    return _assemble(res.results), res


def kernel(x, Wq, Wk, Wv):
    out, _ = _run({"x": x, "Wq": Wq, "Wk": Wk, "Wv": Wv})
    return out


# revision 48
# speedup vs baseline: 1.2177x; 1.0146x over previous
"""Single-head causal attention (B=4, T=2048, C=2048, H=128) on 8 TRN2 cores.

Partial-attention sharding, no inter-core communication: 2 cores per batch.
Core (2b + par) owns the 8 key tiles {128*(2m+par)} of batch b and computes
  - K^T, V^T for its 1024 own key positions only,
  - Q^T for ALL 2048 query positions of the batch,
  - partial attention:   ot = sum_{own k} exp(s) * v,  den = sum_{own k} exp(s)
over every query. The host adds the two cores' partials per batch and divides
(softmax sums commute; the max-shift is skipped since |s| < ~6).

Per-core x.T is column-permuted to [own tiles | sib tiles]; with that order
the kernel is SPMD-identical:
  - K/V project from cols [0, 1024), Q from all cols,
  - attention key tile j covers query cols [128j, 1024) of BOTH halves:
    own-half diag block gets the triangular mask, sib-half diag block a
    per-core scalar (par=0 -> 1.0, par=1 -> 0.0).

Schedule (from trace analysis; phase 1 used to be PE-bound and serial):
  phase 1a projects K/V/Q-OWN only (6 matmuls per c-tile) and needs just
  the own-half columns of x (4.2MB instead of 8.4MB), so it ends ~9us
  earlier; the Q-SIB projection (32 matmuls) is deferred into attention
  half0's ACT-bound gaps, with the sib-half of x streaming lazily in the
  background. Attention half1 then uses the freshly copied Q-sib.

  PSUM as four 2-bank [128,1024] fp32 tags A-D:
    1a: K->A, V->B, Qown->C.  Transposes on B/D.  Qsib accumulates in D
    during half0 iters 0-3 (copied out at iter 4).
    half0: O on B, S rotates [A, C], dens on A.
    half1: O on C, S rotates [D, A, B], dens on D, A.
  den = one ones-matmul per 512 cols over a bf16 A_sum accumulated on the
  DVE. S is emitted 2 key tiles ahead of O; masks sit ahead of the
  deferred A_sum adds on the DVE queue. exp is one wide ACT instruction
  per key tile. Outputs are bf16; O cols [0,512) ship at iter 4.
"""

import numpy as np
import ml_dtypes

B, T, C, H = 4, 2048, 2048, 128
P = 128                 # tile edge
NCT = C // P            # 16 contraction c-tiles
NKT = 8                 # own key tiles per core
NQ = 2048               # query cols per core (own 1024 | sib 1024)
N_CORES = 8
SCALE = float(H) ** -0.5
BF16 = ml_dtypes.bfloat16

WQUARTER = 4            # weight c-tiles per DMA chunk (4 chunks per tensor)
# own-half x chunks: (c-tile list, ring); weight quarter q ships just
# before the x chunk at index WQ_BEFORE[g] (quarter 0 before everything)
XSCHED = [
    ([0], "scalar"), ([1], "sync"), ([2], "scalar"), ([3], "sync"),
    ([4], "scalar"), ([5], "sync"), ([6, 7], "scalar"), ([8, 9], "sync"),
    ([10, 11], "scalar"), ([12, 13], "sync"), ([14, 15], "scalar"),
]
WQ_BEFORE = {3: (1, "sync"), 7: (2, "sync"), 9: (3, "sync")}
# sib-half x chunks (4 c-tiles each), needed only by the deferred Q-sib
# matmuls in attention half0
SIBSCHED = [("scalar", 0), ("sync", 1), ("scalar", 2), ("sync", 3)]

_cache = {}


def _build():
    import concourse.bass as bass
    import concourse.mybir as mybir
    import concourse.tile as tile
    from concourse import bacc
    from concourse.masks import make_identity, make_upper_triangular

    dt = mybir.dt
    nc = bacc.Bacc(
        "TRN2",
        target_bir_lowering=False,
        debug=False,
        enable_asserts=False,
        num_devices=N_CORES,
    )

    xkvT = nc.dram_tensor("xkvT", [C, T], dt.bfloat16, kind="ExternalInput").ap()
    w_d = {
        (nm, q): nc.dram_tensor(
            f"{nm}q{q}", [P, WQUARTER, H], dt.bfloat16, kind="ExternalInput"
        ).ap()
        for nm in ("wq", "wk", "wv")
        for q in range(NCT // WQUARTER)
    }
    # sib-half diag block allowed: 1.0 on par=0 cores, 0.0 on par=1 cores
    odd_d = nc.dram_tensor("odd", [P, 1], dt.float32, kind="ExternalInput").ap()
    ot_d = nc.dram_tensor("ot", [H, NQ], dt.bfloat16, kind="ExternalOutput").ap()
    den_d = nc.dram_tensor("den", [1, NQ], dt.float32, kind="ExternalOutput").ap()

    with tile.TileContext(nc) as tc:
        with (
            tc.tile_pool(name="persist", bufs=1) as persist,
            tc.tile_pool(name="ephem", bufs=8) as ephem,
            tc.tile_pool(name="outp", bufs=2) as outp,
            tc.tile_pool(name="psum", bufs=1, space="PSUM") as psum,
        ):
            # PSUM = four 2-bank [128,1024] fp32 tiles, tags A..D
            def pbank(tag, shape=(P, 1024), dtype=dt.float32, name=None):
                return psum.tile(
                    list(shape), dtype, tag=f"pp{tag}", name=name or f"pp{tag}"
                )

            wq_sb = persist.tile([P, NCT, H], dt.bfloat16)
            wk_sb = persist.tile([P, NCT, H], dt.bfloat16)
            wv_sb = persist.tile([P, NCT, H], dt.bfloat16)
            odd_sb = persist.tile([P, 1], dt.float32)
            xg_sb = [
                persist.tile([P, len(cts), 1024], dt.bfloat16, name=f"xg{g}")
                for g, (cts, _) in enumerate(XSCHED)
            ]
            xs_sb = [
                persist.tile([P, WQUARTER, 1024], dt.bfloat16, name=f"xs{g}")
                for g in range(len(SIBSCHED))
            ]
            k_sb = persist.tile([P, P * NKT], dt.bfloat16)   # K^T own [h, 1024]
            vt_sb = persist.tile([P, P * NKT], dt.bfloat16)  # V^T own [h, 1024]
            v_sb = persist.tile([P, NKT, H], dt.bfloat16)    # V tiles [k, h]
            q_sb = persist.tile([P, NQ], dt.bfloat16)        # Q^T [h, 2048]
            # A_sum in bf16: half the DVE add cost, den matmul reads it
            # directly (den rel err ~0.5%, well inside the 2e-2 gate)
            asum = [
                persist.tile([P, 1024], dt.bfloat16, name=f"asum{h}") for h in (0, 1)
            ]
            ident = persist.tile([P, P], dt.bfloat16)
            tri = persist.tile([P, P], dt.bfloat16)          # 1 where k <= q
            ones_sb = persist.tile([P, 1], dt.bfloat16)

            w_sb = {"wq": wq_sb, "wk": wk_sb, "wv": wv_sb}

            def emit_wq(q, eng):
                lo = WQUARTER * q
                for nm in ("wk", "wv", "wq"):
                    eng.dma_start(
                        out=w_sb[nm][:, lo:lo + WQUARTER, :], in_=w_d[(nm, q)][:]
                    )

            emit_wq(0, nc.sync)
            make_identity(nc, ident[:])
            make_upper_triangular(nc, tri[:], val=1.0, diag=True)
            nc.vector.memset(ones_sb[:], 1.0)

            # ---- phase 1a: own-half x load + K/V/Q-own accumulation ----
            ps_k = pbank("A", name="psk")
            ps_v = pbank("B", name="psv")
            ps_qo = pbank("C", name="psqo")
            for g, (cts, ring) in enumerate(XSCHED):
                eng = nc.scalar if ring == "scalar" else nc.sync
                if g in WQ_BEFORE:
                    q, wring = WQ_BEFORE[g]
                    emit_wq(q, nc.scalar if wring == "scalar" else nc.sync)
                c_lo, w = cts[0], len(cts)
                eng.dma_start(
                    out=xg_sb[g][:],
                    in_=xkvT[P * c_lo:P * (c_lo + w), 0:1024].rearrange(
                        "(j p) t -> p j t", p=P
                    ),
                )
                for jj, j in enumerate(cts):
                    st, sp = j == 0, j == NCT - 1
                    for n in range(2):
                        nc.tensor.matmul(
                            ps_k[:, 512 * n:512 * (n + 1)],
                            lhsT=wk_sb[:, j, :],
                            rhs=xg_sb[g][:, jj, 512 * n:512 * (n + 1)],
                            start=st, stop=sp,
                        )
                    for n in range(2):
                        nc.tensor.matmul(
                            ps_v[:, 512 * n:512 * (n + 1)],
                            lhsT=wv_sb[:, j, :],
                            rhs=xg_sb[g][:, jj, 512 * n:512 * (n + 1)],
                            start=st, stop=sp,
                        )
                    for n in range(2):
                        nc.tensor.matmul(
                            ps_qo[:, 512 * n:512 * (n + 1)],
                            lhsT=wq_sb[:, j, :],
                            rhs=xg_sb[g][:, jj, 512 * n:512 * (n + 1)],
                            start=st, stop=sp,
                        )

            # sib-half x streams in the background (needed from attention
            # half0 iter 0 onward); odd scalar rides along
            for g, (ring, _) in enumerate(SIBSCHED):
                eng = nc.scalar if ring == "scalar" else nc.sync
                eng.dma_start(
                    out=xs_sb[g][:],
                    in_=xkvT[
                        P * WQUARTER * g:P * WQUARTER * (g + 1), 1024:2048
                    ].rearrange("(j p) t -> p j t", p=P),
                )
            nc.sync.dma_start(out=odd_sb[:], in_=odd_d[:])

            # preload the ACT exp table (after the DMA issues so the table
            # load does not delay chunk 0 on the scalar queue)
            warm_sb = persist.tile([P, 1], dt.float32)
            nc.scalar.activation(
                warm_sb[:], ones_sb[:], mybir.ActivationFunctionType.Exp
            )

            # ---- phase boundary: PSUM -> SBUF ----
            # DVE: k then q-own (gates S j=0); ACT: vt (gpsimd has no PSUM
            # port); narrow first k copy so S(0) starts early
            nc.scalar.activation(
                vt_sb[:, 0:512], ps_v[:, 0:512],
                mybir.ActivationFunctionType.Copy,
            )
            nc.vector.tensor_copy(k_sb[:, 0:256], ps_k[:, 0:256])
            nc.vector.tensor_copy(q_sb[:, 0:512], ps_qo[:, 0:512])
            nc.vector.tensor_copy(q_sb[:, 512:1024], ps_qo[:, 512:1024])
            nc.scalar.activation(
                vt_sb[:, 512:1024], ps_v[:, 512:1024],
                mybir.ActivationFunctionType.Copy,
            )
            nc.vector.tensor_copy(k_sb[:, 256:512], ps_k[:, 256:512])
            nc.vector.tensor_copy(k_sb[:, 512:1024], ps_k[:, 512:1024])

            # ---- V tiles via PE transpose (tags B/D ping-pong, keeping A
            # and C free for half0's S rotation) ----
            for kt in range(NKT):
                ps_t = pbank(
                    "BD"[kt % 2], shape=(P, P), dtype=dt.bfloat16, name="pst"
                )
                nc.tensor.transpose(
                    ps_t[:], vt_sb[:, kt * P:(kt + 1) * P], ident[:]
                )
                nc.vector.tensor_copy(v_sb[:, kt, :], ps_t[:])

            # ---- attention ----
            ps_qs = pbank("D", name="psqs")   # deferred Q-sib accumulator

            # Q-sib c-tiles spread thinly over half0 iters so the batches
            # never delay an S emission by more than ~1.3us
            QSIB_SPREAD = [[0, 1, 2], [3, 4, 5], [6, 7, 8], [9, 10, 11],
                           [12, 13], [14, 15], [], []]

            def qsib_mm(it):
                for j in QSIB_SPREAD[it]:
                    st, sp = j == 0, j == NCT - 1
                    for n in range(2):
                        nc.tensor.matmul(
                            ps_qs[:, 512 * n:512 * (n + 1)],
                            lhsT=wq_sb[:, j, :],
                            rhs=xs_sb[j // WQUARTER][:, j % WQUARTER,
                                                     512 * n:512 * (n + 1)],
                            start=st, stop=sp,
                        )

            def den_mm(tag, half, n, den_sb):
                ps_d = pbank(tag, shape=(1, 512), name="psd")
                nc.tensor.matmul(
                    ps_d[:],
                    lhsT=ones_sb[:],
                    rhs=asum[half][:, 512 * n:512 * (n + 1)],
                    start=True, stop=True,
                )
                nc.vector.tensor_copy(den_sb[:, 512 * n:512 * (n + 1)], ps_d[:])

            def attention_half(half, s_tags, o_tag, den_tags, lookahead):
                base = 1024 * half
                ps_o = pbank(o_tag, name=f"pso{half}")
                den_sb = outp.tile([1, 1024], dt.float32, name="den_sb")
                a_tiles = [None] * NKT

                def emit_S(j):
                    # S matmuls + one wide exp for key tile j
                    c0 = P * j
                    ps_s = pbank(s_tags[j % len(s_tags)], name="pss")
                    a_sb = ephem.tile([P, 1024], dt.bfloat16, name="a_sb")
                    a_tiles[j] = a_sb
                    if c0 < 512:
                        nc.tensor.matmul(
                            ps_s[:, c0:512],
                            lhsT=k_sb[:, c0:c0 + P],
                            rhs=q_sb[:, base + c0:base + 512],
                            start=True, stop=True,
                        )
                    lo2 = max(c0, 512)
                    nc.tensor.matmul(
                        ps_s[:, lo2:1024],
                        lhsT=k_sb[:, c0:c0 + P],
                        rhs=q_sb[:, base + lo2:base + 1024],
                        start=True, stop=True,
                    )
                    nc.scalar.activation(
                        a_sb[:, c0:1024], ps_s[:, c0:1024],
                        mybir.ActivationFunctionType.Exp,
                        scale=SCALE,
                    )
                    # diag block mask (DVE, ahead of the asum adds)
                    if half == 0:
                        nc.vector.tensor_mul(
                            a_sb[:, c0:c0 + P], a_sb[:, c0:c0 + P], tri[:]
                        )
                    else:
                        nc.vector.tensor_scalar_mul(
                            a_sb[:, c0:c0 + P], a_sb[:, c0:c0 + P], odd_sb[:]
                        )

                def asum_update(j):
                    # deferred one iteration so the DVE never delays a mask
                    c0 = P * j
                    if j == 0:
                        nc.vector.tensor_copy(asum[half][:, 0:1024], a_tiles[0][:])
                    else:
                        nc.vector.tensor_add(
                            asum[half][:, c0:1024],
                            asum[half][:, c0:1024],
                            a_tiles[j][:, c0:1024],
                        )

                ot_sb = outp.tile([P, 1024], dt.bfloat16, name="ot_sb")
                for l in range(lookahead):
                    emit_S(l)
                for j in range(NKT):
                    # with a 2-tag rotation, +1 lookahead means S(j+1)
                    # tag-waits exp(j-1) (already done) instead of exp(j)
                    if j + lookahead < NKT:
                        emit_S(j + lookahead)
                    c0 = P * j
                    lo2 = max(c0, 512)
                    a_sb = a_tiles[j]
                    # O accumulation (cols 0:512 from j<=3; 512:1024 all j)
                    if c0 < 512:
                        nc.tensor.matmul(
                            ps_o[:, c0:512],
                            lhsT=v_sb[:, j, :],
                            rhs=a_sb[:, c0:512],
                            start=j == 0, stop=j == 3,
                        )
                    nc.tensor.matmul(
                        ps_o[:, lo2:1024],
                        lhsT=v_sb[:, j, :],
                        rhs=a_sb[:, lo2:1024],
                        start=j == 0, stop=j == NKT - 1,
                    )
                    if half == 0 and j < 6:
                        # deferred Q-sib projection as PE filler
                        qsib_mm(j)
                    if half == 0 and j == 6:
                        # Q-sib complete: copy for half1
                        nc.vector.tensor_copy(
                            q_sb[:, 1024:1536], ps_qs[:, 0:512]
                        )
                        nc.vector.tensor_copy(
                            q_sb[:, 1536:2048], ps_qs[:, 512:1024]
                        )
                    if j > 0:
                        asum_update(j - 1)
                    if j == 5:
                        # O cols [0,512) final since j==3: ship them early
                        nc.vector.tensor_copy(ot_sb[:, 0:512], ps_o[:, 0:512])
                        nc.sync.dma_start(
                            out=ot_d[:, base:base + 512], in_=ot_sb[:, 0:512]
                        )

                asum_update(NKT - 1)
                den_mm(den_tags[0], half, 0, den_sb)
                den_mm(den_tags[1], half, 1, den_sb)

                nc.vector.tensor_copy(ot_sb[:, 512:1024], ps_o[:, 512:1024])
                nc.sync.dma_start(
                    out=ot_d[:, base + 512:base + 1024], in_=ot_sb[:, 512:1024]
                )
                nc.sync.dma_start(out=den_d[:, base:base + 1024], in_=den_sb[:])

            attention_half(0, s_tags=["A", "C"], o_tag="B",
                           den_tags=("A", "A"), lookahead=1)
            attention_half(1, s_tags=["D", "A", "B"], o_tag="C",
                           den_tags=("D", "A"), lookahead=2)

    nc.compile()
    return nc


def _core_perm(core):
    par = core % 2
    own = [2 * m + par for m in range(NKT)]
    sib = [2 * m + 1 - par for m in range(NKT)]
    return own + sib


def _prep_inputs(x, Wq, Wk, Wv):
    """Build the 8 per-core input maps."""
    def wshape(w):
        # [C, H] -> [128, NCT, H]: w_r[p, j, h] = w[j*128 + p, h]
        return np.ascontiguousarray(
            w.astype(BF16).reshape(NCT, P, H).transpose(1, 0, 2)
        )

    wmap = {}
    for nm, w in (("wq", Wq), ("wk", Wk), ("wv", Wv)):
        wb = wshape(w)
        for q in range(NCT // WQUARTER):
            wmap[f"{nm}q{q}"] = np.ascontiguousarray(
                wb[:, WQUARTER * q:WQUARTER * (q + 1), :]
            )
    x_bf = x.astype(BF16)

    in_maps = []
    for core in range(N_CORES):
        b, par = core // 2, core % 2
        cols = np.concatenate(
            [np.arange(P * t, P * t + P) for t in _core_perm(core)]
        )
        xT = np.ascontiguousarray(x_bf[b].T[:, cols])
        odd = np.full((P, 1), 1.0 - par, np.float32)
        in_maps.append({
            "xkvT": xT,
            **wmap,
            "odd": np.ascontiguousarray(odd),
        })
    return in_maps


def _assemble(results):
    num = np.zeros((B, T, H), np.float32)
    den = np.zeros((B, T, 1), np.float32)
    for core in range(N_CORES):
        b = core // 2
        r = results[core]
        oT = r["ot"].astype(np.float32).T   # [2048, H]
        dT = r["den"].T                     # [2048, 1]
        for i, g in enumerate(_core_perm(core)):
            num[b, P * g:P * (g + 1)] += oT[P * i:P * (i + 1)]
            den[b, P * g:P * (g + 1)] += dT[P * i:P * (i + 1)]
    return num / den


def _run(inputs, trace=False, **spmd_kwargs):
    from concourse.bass_utils import run_bass_kernel_spmd

    if "nc" not in _cache:
        _cache["nc"] = _build()
    nc = _cache["nc"]
    in_maps = _prep_inputs(
        np.asarray(inputs["x"], np.float32),
        np.asarray(inputs["Wq"], np.float32),
        np.asarray(inputs["Wk"], np.float32),
        np.asarray(inputs["Wv"], np.float32),
    )
    res = run_bass_kernel_spmd(
        nc, in_maps, list(range(N_CORES)), trace=trace, **spmd_kwargs
    )
    return _assemble(res.results), res


def kernel(x, Wq, Wk, Wv):
    out, _ = _run({"x": x, "Wq": Wq, "Wk": Wk, "Wv": Wv})
    return out
